# revision 7
# baseline (speedup 1.0000x reference)
"""Self-contained Trainium2 Bass kernel for a single attention head.

Reference computation (per batch b):
    Q = x @ Wq + bq ; K = x @ Wk + bk ; V = x @ Wv + bv      (x: [S, M])
    out = softmax(Q K^T / sqrt(D)) @ V                        ([S, D])

Shapes: B=4, S=4096, M=1024, D=128, f32.

Sharding: 8 cores; core c handles batch b=c//2, query-half h=c%2 (2048 query
rows), with the full batch (4096 rows) as keys/values. Softmax is over the
key axis only, so key order is irrelevant: the host permutes each core's
batch so its own query rows come first, and pre-transposes to xT [M, S] so
the device needs no input transposes. No collectives.

Device layout (per core):
  - projections contract over M with fp32r matmuls: Q^T, K^T produced
    dk-major [128, s]; V produced naturally [s, 128] via 128x128 transposes.
  - scores computed transposed: S^T[s, q] = (K^T tile).T @ Q^T, fp32r,
    moving dim 512. exp (ACT engine) writes A^T bf16 - which is exactly the
    layout attn@V needs, so no O(S*S) transposes.
  - softmax denominator: DVE add-chain over the 32 A^T tiles -> [128, q]
    partials, one tiny f32 ones-matmul -> [1, q], transpose+reciprocal ->
    per-q-row scale applied during the final O^T -> O transpose copy.
  - attn@V: O^T[dv, q] accumulated in PSUM over 32 bf16 matmuls.
"""

from contextlib import ExitStack

import numpy as np

import concourse.bass as bass
import concourse.tile as tile
from concourse import bacc, mybir
from concourse.bass_utils import run_bass_kernel_spmd
from concourse.masks import make_identity

F32 = mybir.dt.float32
F32R = mybir.dt.float32r
BF16 = mybir.dt.bfloat16

B, S, M, D = 4, 4096, 1024, 128
N_CORES = 8
SCALE = 1.0 / np.sqrt(np.float32(D))


def build_attention(nc, S_keys=S, S_q=S // 2, M_dim=M, SC=512, QC=512):
    """Emit the attention graph. S_keys: key rows; S_q: query rows (prefix of
    the permuted sequence); SC: phase-1 s-chunk; QC: phase-2 q-chunk."""
    P = 128
    MT = M_dim // P              # m-tiles
    ST = S_keys // P             # key s-tiles
    NSC = S_keys // SC           # phase-1 chunks
    NSCQ = S_q // SC             # phase-1 chunks that also need Q
    NQC = S_q // QC              # phase-2 q-chunks
    SCT = SC // P                # 128-tiles per s-chunk
    QT = QC // P                 # 128-tiles per q-chunk

    xT = nc.dram_tensor("xT", [M_dim, S_keys], F32, kind="ExternalInput").ap()
    wq = nc.dram_tensor("wq", [M_dim, D], F32, kind="ExternalInput").ap()
    wk = nc.dram_tensor("wk", [M_dim, D], F32, kind="ExternalInput").ap()
    wv = nc.dram_tensor("wv", [M_dim, D], F32, kind="ExternalInput").ap()
    bq = nc.dram_tensor("bq", [D, 1], F32, kind="ExternalInput").ap()
    bk = nc.dram_tensor("bk", [D, 1], F32, kind="ExternalInput").ap()
    bv = nc.dram_tensor("bv", [D, 1], F32, kind="ExternalInput").ap()
    out = nc.dram_tensor("out", [S_q, D], F32, kind="ExternalOutput").ap()

    xT_r = xT.rearrange("(t p) s -> p t s", p=P)
    out_r = out.rearrange("(t p) d -> p t d", p=P)

    with tile.TileContext(nc) as tc, ExitStack() as ctx:
        persist = ctx.enter_context(tc.tile_pool(name="persist", bufs=1))

        ident = persist.tile([P, P], F32)
        make_identity(nc, ident[:])
        ones_col = persist.tile([P, 1], F32)
        nc.vector.memset(ones_col[:], 1.0)

        # weights: DMA f32 then round to fp32r
        w_r = []
        b_sb = []
        with tc.tile_pool(name="wstage", bufs=1) as wstage:
            for name, w_ap, b_ap in (("q", wq, bq), ("k", wk, bk), ("v", wv, bv)):
                st_w = wstage.tile([P, MT, D], F32)
                nc.sync.dma_start(st_w[:], w_ap.rearrange("(t p) d -> p t d", p=P))
                wr = persist.tile([P, MT, D], F32R, name=f"w{name}_r")
                nc.any.tensor_copy(wr[:], st_w[:])
                w_r.append(wr)
                bs = persist.tile([P, 1], F32, name=f"b{name}_sb")
                nc.sync.dma_start(bs[:], b_ap)
                b_sb.append(bs)
        wq_r, wk_r, wv_r = w_r
        bq_sb, bk_sb, bv_sb = b_sb

        kT_sb = persist.tile([P, S_keys], F32R)   # K^T  [dk, s]
        qT_sb = persist.tile([P, S_q], F32R)      # Q^T  [dk, q]
        v_sb = persist.tile([P, ST, D], BF16)     # V    [s%128, s-tile, dv]
        o_sb = persist.tile([P, S_q // P, D], F32)  # O   [q%128, q-tile, dv]

        Ident = mybir.ActivationFunctionType.Identity
        Exp = mybir.ActivationFunctionType.Exp
        Copy = mybir.ActivationFunctionType.Copy

        # ---- phase 1: projections ----
        with (
            tc.tile_pool(name="xstage", bufs=2) as xstage,
            tc.tile_pool(name="xr", bufs=2) as xrpool,
            tc.tile_pool(name="vtmp", bufs=2) as vtmp,
            tc.tile_pool(name="p1psum", bufs=2, space="PSUM") as p1psum,
            tc.tile_pool(name="p1tpsum", bufs=2, space="PSUM") as p1tpsum,
        ):
            for sc in range(NSC):
                ssl = bass.ds(sc * SC, SC)
                x_f = xstage.tile([P, MT, SC], F32)
                nc.sync.dma_start(x_f[:], xT_r[:, :, ssl])
                x_r = xrpool.tile([P, MT, SC], F32R)
                nc.any.tensor_copy(x_r[:], x_f[:])

                # K^T chunk
                ps_k = p1psum.tile([P, SC], F32)
                for mt in range(MT):
                    nc.tensor.matmul(ps_k[:], wk_r[:, mt, :], x_r[:, mt, :],
                                     start=(mt == 0), stop=(mt == MT - 1))
                nc.scalar.activation(kT_sb[:, ssl], ps_k[:], Ident, bias=bk_sb[:])

                # Q^T chunk (query rows are the permuted prefix)
                if sc < NSCQ:
                    ps_q = p1psum.tile([P, SC], F32)
                    for mt in range(MT):
                        nc.tensor.matmul(ps_q[:], wq_r[:, mt, :], x_r[:, mt, :],
                                         start=(mt == 0), stop=(mt == MT - 1))
                    nc.scalar.activation(qT_sb[:, ssl], ps_q[:], Ident, bias=bq_sb[:])

                # V^T chunk, then transpose to natural V tiles
                ps_v = p1psum.tile([P, SC], F32)
                for mt in range(MT):
                    nc.tensor.matmul(ps_v[:], wv_r[:, mt, :], x_r[:, mt, :],
                                     start=(mt == 0), stop=(mt == MT - 1))
                vt = vtmp.tile([P, SC], F32)
                nc.scalar.activation(vt[:], ps_v[:], Ident, bias=bv_sb[:])
                for t in range(SCT):
                    ps_t = p1tpsum.tile([P, D], F32)
                    nc.tensor.transpose(ps_t[:], vt[:, bass.ts(t, P)], ident[:])
                    nc.any.tensor_copy(v_sb[:, sc * SCT + t, :], ps_t[:])

        # ---- phase 2: attention ----
        with (
            tc.tile_pool(name="a_sb", bufs=2) as apool,
            tc.tile_pool(name="dacc", bufs=2) as dpool,
            tc.tile_pool(name="small", bufs=2 * QT) as small,
            tc.tile_pool(name="otmp", bufs=2) as otpool,
            tc.tile_pool(name="spsum", bufs=3, space="PSUM") as spsum,
            tc.tile_pool(name="opsum", bufs=1, space="PSUM") as opsum,
            tc.tile_pool(name="dpsum", bufs=1, space="PSUM") as dpsum,
            tc.tile_pool(name="otpsum", bufs=2, space="PSUM") as otpsum,
        ):
            for qc in range(NQC):
                qsl = bass.ds(qc * QC, QC)
                a_sb = apool.tile([P, ST, QC], BF16)
                den = dpool.tile([P, QC], F32)

                # pass 1: scores + exp + denominator partials
                for st in range(ST):
                    ps_s = spsum.tile([P, QC], F32)
                    nc.tensor.matmul(ps_s[:], kT_sb[:, bass.ts(st, P)],
                                     qT_sb[:, qsl], start=True, stop=True)
                    nc.scalar.activation(a_sb[:, st, :], ps_s[:], Exp, scale=float(SCALE))
                    if st == 0:
                        nc.vector.tensor_copy(den[:], a_sb[:, 0, :])
                    else:
                        nc.vector.tensor_add(den[:], den[:], a_sb[:, st, :])

                # pass 2: O^T accumulation
                ps_o = opsum.tile([P, QC], F32)
                for st in range(ST):
                    nc.tensor.matmul(ps_o[:], v_sb[:, st, :], a_sb[:, st, :],
                                     start=(st == 0), stop=(st == ST - 1))
                oT = otpool.tile([P, QC], F32)
                nc.any.tensor_copy(oT[:], ps_o[:])

                # denominator: [128, QC] -> [1, QC] -> transpose -> reciprocal
                ps_d = dpsum.tile([1, QC], F32)
                nc.tensor.matmul(ps_d[:], ones_col[:], den[:], start=True, stop=True)
                den_flat = small.tile([1, QC], F32)
                nc.any.tensor_copy(den_flat[:], ps_d[:])

                for t in range(QT):
                    ps_dt = dpsum.tile([P, 1], F32)
                    nc.tensor.transpose(ps_dt[:], den_flat[:1, bass.ts(t, P)],
                                        ident[:1, :1])
                    rden = small.tile([P, 1], F32)
                    nc.vector.reciprocal(rden[:], ps_dt[:])
                    ps_ot = otpsum.tile([P, D], F32)
                    nc.tensor.transpose(ps_ot[:], oT[:, bass.ts(t, P)], ident[:])
                    nc.scalar.activation(o_sb[:, qc * QT + t, :], ps_ot[:], Copy,
                                         scale=rden[:])

            nc.sync.dma_start(out_r[:], o_sb[:])

    return nc


def build(n_cores=N_CORES, **kw):
    nc = bacc.Bacc("TRN2", target_bir_lowering=False, debug=False,
                   num_devices=n_cores)
    build_attention(nc, **kw)
    nc.compile()
    return nc


def shard_inputs(input, Wq, bq, Wk, bk, Wv, bv):
    """Build per-core in_maps. Core c: batch c//2, query-half c%2, with the
    core's query rows permuted to the front (softmax is key-permutation
    invariant)."""
    half = S // 2
    in_maps = []
    for c in range(N_CORES):
        b, h = divmod(c, 2)
        xb = np.asarray(input[b])
        x_perm = np.concatenate(
            [xb[h * half:(h + 1) * half], xb[(1 - h) * half:(2 - h) * half]], axis=0
        )
        in_maps.append({
            "xT": np.ascontiguousarray(x_perm.T, dtype=np.float32),
            "wq": np.asarray(Wq, dtype=np.float32),
            "wk": np.asarray(Wk, dtype=np.float32),
            "wv": np.asarray(Wv, dtype=np.float32),
            "bq": np.asarray(bq, dtype=np.float32).reshape(D, 1),
            "bk": np.asarray(bk, dtype=np.float32).reshape(D, 1),
            "bv": np.asarray(bv, dtype=np.float32).reshape(D, 1),
        })
    return in_maps


_NC_CACHE = {}


def kernel(input, Wq, bq, Wk, bk, Wv, bv):
    in_maps = shard_inputs(input, Wq, bq, Wk, bk, Wv, bv)
    if "nc" not in _NC_CACHE:
        _NC_CACHE["nc"] = build()
    nc = _NC_CACHE["nc"]
    res = run_bass_kernel_spmd(nc, in_maps, core_ids=list(range(N_CORES)))
    half = S // 2
    result = np.empty((B, S, D), dtype=np.float32)
    for c in range(N_CORES):
        b, h = divmod(c, 2)
        result[b, h * half:(h + 1) * half] = res.results[c]["out"]
    return result


if __name__ == "__main__":
    rng = np.random.default_rng(0)
    inputs = {
        "input": rng.standard_normal((B, S, M), dtype=np.float32),
        "Wq": (rng.standard_normal((M, D), dtype=np.float32) / np.sqrt(M)).astype(np.float32),
        "bq": (rng.standard_normal(D, dtype=np.float32) * 0.02),
        "Wk": (rng.standard_normal((M, D), dtype=np.float32) / np.sqrt(M)).astype(np.float32),
        "bk": (rng.standard_normal(D, dtype=np.float32) * 0.02),
        "Wv": (rng.standard_normal((M, D), dtype=np.float32) / np.sqrt(M)).astype(np.float32),
        "bv": (rng.standard_normal(D, dtype=np.float32) * 0.02),
    }
    out = kernel(**inputs)
    print("kernel output:", out.shape, out.dtype)
